# revision 1
# baseline (speedup 1.0000x reference)
"""Multi-head attention block on 8 Trainium2 NeuronCores.

Problem: B=8, N=1024, E=768, H=12, D=64 attention (QKV proj -> softmax(QK^T/8)V
-> output proj), fp32 I/O.

Sharding: data parallel over batch — core b computes batch element b entirely
locally; no collectives. Host shards/stacks.

Per-core kernel (matmuls in fp32r — hardware TF32-like mode, 1 cyc/row):
  phase 0: DMA x -> PE-transpose (batched 4 per psum tile) -> xT [E, N]
  phase 1: V natural [N, 65*12] with a ones column per head (col 65h+64),
           then qT/kT pairs [128, N]: rows (h%2)*64 hold head h's d-dims.
           Q/K bias via per-partition scalar add at psum evac; V/proj bias
           via ones-outer-product broadcast tiles added at evac.
  phase 2: per head: S^T[k,q] = K^T.T Q^T (two 512-wide matmuls into one
           [128,1024] psum); exp on ACT (scale=1/8, one 1024-wide op);
           U_aug[65, q] = [V | 1].T expS (row 64 = softmax denominator Z);
           invZ = 1/Z (f32r); K=1 matmul broadcasts invZ to 64 rows;
           attnT pair tile rows 0:64 (even head, DVE) / 64:128 (odd head,
           DVE -> staging -> partition-shift SBUF DMA)
  phase 3: out[t, e] = sum_c attnT[c].T W_proj[c] + b_proj (128-contraction)
"""
import numpy as np

B, N, E, H, D = 8, 1024, 768, 12, 64
SCALE = D ** -0.5
NT = N // 128   # token chunks (8)
NE = E // 128   # embed chunks (6)
NQ = N // 512   # moving-dim tiles (2)
NFS = [(0, 512), (512, 256)]  # free-dim split of E for matmuls


def _build():
    import concourse.bacc as bacc
    import concourse.mybir as mybir
    import concourse.tile as tile
    from concourse.masks import make_identity

    F32 = mybir.dt.float32
    F32R = mybir.dt.float32r
    EXP = mybir.ActivationFunctionType.Exp

    nc = bacc.Bacc("TRN2", target_bir_lowering=False)
    x_d = nc.declare_dram_parameter("x", [N, E], F32, isOutput=False)
    wqkv_d = nc.declare_dram_parameter("W_qkv", [E, 3 * E], F32, isOutput=False)
    bqkv_d = nc.declare_dram_parameter("b_qkv", [3 * E], F32, isOutput=False)
    wproj_d = nc.declare_dram_parameter("W_proj", [E, E], F32, isOutput=False)
    bproj_d = nc.declare_dram_parameter("b_proj", [E], F32, isOutput=False)
    out_d = nc.declare_dram_parameter("out", [N, E], F32, isOutput=True)

    with tile.TileContext(nc) as tc:
        with (
            tc.tile_pool(name="const", bufs=1) as cp,
            tc.tile_pool(name="qkv", bufs=1) as qp,
            tc.tile_pool(name="psum", bufs=1, space="PSUM") as ps,
        ):
            # ---- constants ----
            identf = cp.tile([128, 128], F32)
            make_identity(nc, identf)
            ident = cp.tile([128, 128], F32R)
            nc.vector.tensor_copy(ident, identf)
            ones1f = cp.tile([1, 128], F32)
            nc.vector.memset(ones1f, 1.0)
            ones1 = cp.tile([1, 128], F32R)
            nc.vector.tensor_copy(ones1, ones1f)
            ones65f = cp.tile([65, 64], F32)
            nc.vector.memset(ones65f, 1.0)
            ones65 = cp.tile([65, 64], F32R)
            nc.vector.tensor_copy(ones65, ones65f)
            bq_cols = [cp.tile([128, 1], F32, name=f"bq_{fc}", tag=f"bq_{fc}")
                       for fc in range(12)]

            # ---- long-lived attention-layout tensors ----
            qT = [qp.tile([128, N], F32R, name=f"qT{c}", tag=f"qT{c}")
                  for c in range(6)]
            kT = [qp.tile([128, N], F32R, name=f"kT{c}", tag=f"kT{c}")
                  for c in range(6)]
            vS = [qp.tile([128, 65 * H], F32R, name=f"vS{i}", tag=f"vS{i}")
                  for i in range(NT)]
            attnT = [qp.tile([128, N], F32R, name=f"attnT{p}", tag=f"attnT{p}")
                     for p in range(6)]

            from contextlib import ExitStack
            with ExitStack() as _xs:
                xp = tc.alloc_tile_pool(name="xw", bufs=1)
                xtp = tc.alloc_tile_pool(name="xtp", bufs=1)
                _xs.callback(lambda: xp.release())
                # ---- phase 0: load x (SWDGE-first = bandwidth priority),
                # transpose 8 per 2-bank psum tile (one group per j) ----
                xT = [xp.tile([128, N], F32R, name=f"xT{j}", tag=f"xT{j}")
                      for j in range(NE)]
                xts = {}
                for i in range(NT):
                    xt_i = xtp.tile([128, E], F32, name=f"xt{i}", tag=f"xt{i}")
                    nc.sync.dma_start(
                        out=xt_i, in_=x_d[i * 128:(i + 1) * 128, :])
                    xts[i] = xt_i
                # Q/K bias columns, queued on HWDGE after the x tiles
                for fc in range(12):
                    nc.sync.dma_start(
                        out=bq_cols[fc],
                        in_=bqkv_d[fc * 128:(fc + 1) * 128].rearrange(
                            "(p o) -> p o", o=1))
                # weights: V-bias row first, then V columns, then QK columns
                # SWDGE queue order gives x transfer priority over weights
                bv_row = xp.tile([1, E], F32R)
                nc.gpsimd.dma_start(
                    out=bv_row,
                    in_=bqkv_d[2 * E:3 * E].rearrange("(o f) -> o f", o=1))
                wqv = [xp.tile([128, E], F32R, name=f"wqv{j}", tag=f"wqv{j}")
                       for j in range(NE)]
                for j in range(NE):
                    nc.gpsimd.dma_start(
                        out=wqv[j], in_=wqkv_d[j * 128:(j + 1) * 128, 2 * E:])
                wqk = [xp.tile([128, 2 * E], F32R, name=f"wqk{j}", tag=f"wqk{j}")
                       for j in range(NE)]
                for j in range(NE):
                    nc.gpsimd.dma_start(
                        out=wqk[j], in_=wqkv_d[j * 128:(j + 1) * 128, 0:2 * E])
                for ig in range(2):
                    for j in range(NE):
                        pt = ps.tile([128, 512], F32, name=f"pt{ig}_{j}",
                                     tag=("s2", "mm", "u")[(ig * NE + j) % 3],
                                     bufs=2)
                        for ii in range(4):
                            i = ig * 4 + ii
                            nc.tensor.transpose(
                                pt[:, ii * 128:(ii + 1) * 128],
                                xts[i][:, j * 128:(j + 1) * 128], identf)
                        nc.vector.tensor_copy(
                            xT[j][:, ig * 512:(ig + 1) * 512], pt)

                xtp.release()
                # ---- phase 1a: V token-major with ones cols + bias ----
                onesH = xp.tile([128, H], F32)
                nc.vector.memset(onesH, 1.0)
                bv_bc = xp.tile([128, E], F32)
                for nf, (f0, fw) in enumerate(NFS):
                    pbv = ps.tile([128, 512], F32, name=f"pbv{nf}", tag="mm",
                                  bufs=2)
                    nc.tensor.matmul(pbv[:, :fw], ones1, bv_row[:, f0:f0 + fw],
                                     start=True, stop=True)
                    nc.vector.tensor_copy(bv_bc[:, f0:f0 + fw], pbv[:, :fw])
                for i in range(NT):
                    nc.vector.tensor_copy(
                        vS[i].rearrange("p (h c) -> p h c", c=65)[:, :, 64:65],
                        onesH.rearrange("p (h o) -> p h o", o=1))
                    for nf, (f0, fw) in enumerate(NFS):
                        pv = ps.tile([128, 512], F32, name=f"pv{i}_{nf}",
                                     tag=("s2", "mm", "u")[(i * 2 + nf) % 3],
                                     bufs=2)
                        for j in range(NE):
                            nc.tensor.matmul(
                                pv[:, :fw],
                                xT[j][:, i * 128:(i + 1) * 128],
                                wqv[j][:, f0:f0 + fw],
                                start=(j == 0), stop=(j == NE - 1))
                        nh, h0 = fw // D, f0 // D
                        nc.vector.tensor_add(
                            vS[i].rearrange("p (h c) -> p h c", c=65)
                                [:, h0:h0 + nh, 0:64],
                            pv[:, :fw].rearrange("p (h d) -> p h d", d=D),
                            bv_bc[:, f0:f0 + fw].rearrange(
                                "p (h d) -> p h d", d=D))

                # ---- phase 1b: Q^T / K^T feature-major pairs + bias ----
                for c in range(12):  # 0..5 -> qT, 6..11 -> kT
                    dst = qT[c] if c < 6 else kT[c - 6]
                    wcol0 = c * 128
                    for q in range(NQ):
                        pq = ps.tile([128, 512], F32, name=f"pq{c}_{q}",
                                     tag="mm", bufs=2)
                        for j in range(NE):
                            nc.tensor.matmul(
                                pq,
                                wqk[j][:, wcol0:wcol0 + 128],
                                xT[j][:, q * 512:(q + 1) * 512],
                                start=(j == 0), stop=(j == NE - 1))
                        nc.vector.tensor_scalar_add(
                            dst[:, q * 512:(q + 1) * 512], pq, bq_cols[c])

            # ---- phases 2+3: proj pool first so W_proj loads overlap
            # attention; exp pool released before proj matmuls need space ----
            with tc.tile_pool(name="proj", bufs=1) as pp:
                wp_sb = [pp.tile([128, E], F32R, name=f"wp{c}", tag=f"wp{c}")
                         for c in range(6)]
                for c in range(6):
                    nc.gpsimd.dma_start(
                        out=wp_sb[c], in_=wproj_d[c * 128:(c + 1) * 128, :])
                bp_row = pp.tile([1, E], F32R)
                nc.gpsimd.dma_start(
                    out=bp_row, in_=bproj_d[:].rearrange("(o f) -> o f", o=1))
                bp_bc = pp.tile([128, E], F32)
                for nf, (f0, fw) in enumerate(NFS):
                    pbp = ps.tile([128, 512], F32, name=f"pbp{nf}", tag="mm",
                                  bufs=2)
                    nc.tensor.matmul(pbp[:, :fw], ones1, bp_row[:, f0:f0 + fw],
                                     start=True, stop=True)
                    nc.vector.tensor_copy(bp_bc[:, f0:f0 + fw], pbp[:, :fw])
                _run_attention_and_proj(
                    nc, tc, ps, mybir, qT, kT, vS, attnT, ones65,
                    wp_sb, bp_bc, out_d)
    nc.compile()
    return nc


def _run_attention_and_proj(nc, tc2, ps, mybir, qT, kT, vS, attnT, ones65,
                            wp_sb, bp_bc, out_d):
    F32 = mybir.dt.float32
    F32R = mybir.dt.float32r
    EXP = mybir.ActivationFunctionType.Exp
    if True:
            with tc2.tile_pool(name="exp", bufs=1) as ep:
                expS_of = {}

                def emit_S(h):
                    c, r0 = h // 2, (h % 2) * 64
                    expS = [
                        ep.tile([128, N], F32R, name=f"expS{h}_{kc}",
                                tag="expS", bufs=16)
                        for kc in range(NT)]
                    expS_of[h] = expS
                    for kc in range(NT):
                        pss = ps.tile([128, N], F32, name=f"ps{h}_{kc}",
                                      tag="s2", bufs=2)
                        for q in range(NQ):
                            nc.tensor.matmul(
                                pss[:, q * 512:(q + 1) * 512],
                                kT[c][r0:r0 + 64, kc * 128:(kc + 1) * 128],
                                qT[c][r0:r0 + 64, q * 512:(q + 1) * 512],
                                start=True, stop=True)
                        nc.scalar.activation(expS[kc], pss, EXP,
                                             scale=float(SCALE))

                def emit_U(h):
                    c = h // 2
                    expS = expS_of.pop(h)
                    for q in range(NQ):
                        pu = ps.tile([65, 512], F32, name=f"pu{h}_{q}",
                                     tag="u", bufs=2)
                        for kc in range(NT):
                            nc.tensor.matmul(
                                pu,
                                vS[kc][:, h * 65:h * 65 + 65],
                                expS[kc][:, q * 512:(q + 1) * 512],
                                start=(kc == 0), stop=(kc == NT - 1))
                        rz = ep.tile([65, 512], F32R, name=f"rz{h}_{q}",
                                     tag="rz", bufs=2)
                        with nc.allow_low_precision(reason="invZ f32r bcast"):
                            nc.vector.reciprocal(rz[64:65, :], pu[64:65, :])
                        pb = ps.tile([128, 512], F32, name=f"pb{h}_{q}",
                                     tag="mm", bufs=2)
                        nc.tensor.matmul(
                            pb[0:64, :], ones65[64:65, :], rz[64:65, :],
                            start=True, stop=True)
                        pbs = ep.tile([64, 512], F32, name=f"pbs{h}_{q}",
                                      tag="pbs", bufs=2)
                        nc.vector.tensor_copy(pbs, pb[0:64, :])
                        if h % 2 == 0:
                            nc.vector.tensor_mul(
                                attnT[c][0:64, q * 512:(q + 1) * 512],
                                pu[0:64, :], pbs)
                        else:
                            tmp = ep.tile([64, 512], F32R, name=f"tmp{h}_{q}",
                                          tag="tmp", bufs=2)
                            nc.vector.tensor_mul(tmp, pu[0:64, :], pbs)
                            nc.sync.dma_start(
                                out=attnT[c][64:128, q * 512:(q + 1) * 512],
                                in_=tmp)


                for h in range(H):
                    emit_S(h)
                    if h > 0:
                        emit_U(h - 1)
                emit_U(H - 1)

            # ---- phase 3: output projection ----
            with tc2.tile_pool(name="osb", bufs=1) as op:
                for i in range(NT):
                    o_sb = op.tile([128, E], F32, name=f"o{i}", tag="o", bufs=4)
                    for nf, (f0, fw) in enumerate(NFS):
                        po = ps.tile([128, 512], F32, name=f"po{i}_{nf}",
                                     tag=("s2", "mm", "u")[(i * 2 + nf) % 3],
                                     bufs=2)
                        for c in range(6):
                            nc.tensor.matmul(
                                po[:, :fw],
                                attnT[c][:, i * 128:(i + 1) * 128],
                                wp_sb[c][:, f0:f0 + fw],
                                start=(c == 0), stop=(c == 5))
                        nc.vector.tensor_add(
                            o_sb[:, f0:f0 + fw], po[:, :fw],
                            bp_bc[:, f0:f0 + fw])
                    nc.sync.dma_start(
                        out=out_d[i * 128:(i + 1) * 128, :], in_=o_sb)


_NC_CACHE = None


def kernel(x, W_qkv, b_qkv, W_proj, b_proj):
    from concourse.bass_utils import run_bass_kernel_spmd

    global _NC_CACHE
    if _NC_CACHE is None:
        _NC_CACHE = _build()
    nc = _NC_CACHE

    x = np.ascontiguousarray(np.asarray(x, dtype=np.float32))
    W_qkv = np.ascontiguousarray(np.asarray(W_qkv, dtype=np.float32))
    b_qkv = np.ascontiguousarray(np.asarray(b_qkv, dtype=np.float32))
    W_proj = np.ascontiguousarray(np.asarray(W_proj, dtype=np.float32))
    b_proj = np.ascontiguousarray(np.asarray(b_proj, dtype=np.float32))

    in_maps = [
        {"x": x[b], "W_qkv": W_qkv, "b_qkv": b_qkv,
         "W_proj": W_proj, "b_proj": b_proj}
        for b in range(B)
    ]
    res = run_bass_kernel_spmd(nc, in_maps, core_ids=list(range(B)))
    return np.stack([np.asarray(res.results[b]["out"]) for b in range(B)])



# revision 35
# speedup vs baseline: 1.0865x; 1.0865x over previous
"""Multi-head attention block on 8 Trainium2 NeuronCores.

Problem: B=8, N=1024, E=768, H=12, D=64 attention (QKV proj -> softmax(QK^T/8)V
-> output proj), fp32 I/O.

Sharding: data parallel over batch — core b computes batch element b entirely
locally; no collectives. Host shards/stacks.

Per-core kernel v2 (ACT-exp-chain saturated; PE work minimized):
  phase 0: DMA x -> PE-transpose (f32r, 8 per 2-bank psum) -> xT [E, N]
  phase 1 (interleaved with phase 2 per head-pair):
    V natural vS[i] [128, 65*12] bf16 with a ones column per head (col 65h+64)
    qT/kT pairs [128, N] bf16: rows (h%2)*64 hold head h's d-dims; W_qkv
    columns DMA'd as per-pair "stage" tiles [e-part, (j f)] so QK(c) can
    start as soon as its two stages land.
  phase 2 per head: S^T[k,q] psum via 2 512-wide matmuls (bf16);
    exp on ACT (scale=1/8) -> expS bf16; AV: stat=expS chunk, mov=[V] 64-wide
    bf16 accumulating over kc -> attn natural [q,64] psum; Z via 1-wide ones
    matmuls -> pz; invZ=recip(pz); DVE scale -> attnS bf16; PE transpose
    (bf16 identity, 1cyc/row) -> attnT pair psum; DVE evac per pair.
  phase 3: out[t, e] = sum_c attnT[c].T W_proj[c] + b_proj (f32r)
"""
import numpy as np

B, N, E, H, D = 8, 1024, 768, 12, 64
SCALE = D ** -0.5
NT = N // 128   # token chunks (8)
NE = E // 128   # embed chunks (6)
NQ = N // 512   # moving-dim tiles (2)
NFS = [(0, 512), (512, 256)]  # free-dim split of E for matmuls


def _build():
    import concourse.bacc as bacc
    import concourse.mybir as mybir
    import concourse.tile as tile
    from concourse.masks import make_identity
    from contextlib import ExitStack

    F32 = mybir.dt.float32
    F32R = mybir.dt.float32r
    BF16 = mybir.dt.bfloat16
    EXP = mybir.ActivationFunctionType.Exp

    nc = bacc.Bacc("TRN2", target_bir_lowering=False)
    x_d = nc.declare_dram_parameter("x", [N, E], F32, isOutput=False)
    wqkv_d = nc.declare_dram_parameter("W_qkv", [E, 3 * E], F32, isOutput=False)
    bqkv_d = nc.declare_dram_parameter("b_qkv", [3 * E], F32, isOutput=False)
    wproj_d = nc.declare_dram_parameter("W_proj", [E, E], F32, isOutput=False)
    bproj_d = nc.declare_dram_parameter("b_proj", [E], F32, isOutput=False)
    out_d = nc.declare_dram_parameter("out", [N, E], F32, isOutput=True)

    with tile.TileContext(nc) as tc:
        with (
            tc.tile_pool(name="const", bufs=1) as cp,
            tc.tile_pool(name="qkv", bufs=1) as qp,
            tc.tile_pool(name="psum", bufs=1, space="PSUM") as ps,
            tc.tile_pool(name="proj", bufs=1) as pp,
        ):
            # ---- constants ----
            identf = cp.tile([128, 128], F32)
            make_identity(nc, identf)
            ident_b = cp.tile([128, 128], BF16)
            nc.vector.tensor_copy(ident_b, identf)
            ones1 = cp.tile([1, 128], BF16)
            nc.vector.memset(ones1, 1.0)
            onesH = cp.tile([128, H], BF16)
            nc.vector.memset(onesH, 1.0)
            bq_cols = [cp.tile([128, 1], F32, name=f"bq_{fc}", tag=f"bq_{fc}")
                       for fc in range(12)]

            # ---- long-lived attention-layout tensors ----
            qT = [qp.tile([128, N], BF16, name=f"qT{c}", tag=f"qT{c}")
                  for c in range(6)]
            kT = [qp.tile([128, N], BF16, name=f"kT{c}", tag=f"kT{c}")
                  for c in range(6)]
            vS = [qp.tile([128, 65 * H], BF16, name=f"vS{i}", tag=f"vS{i}")
                  for i in range(NT)]
            attnT = [qp.tile([128, N], BF16, name=f"attnT{p}", tag=f"attnT{p}")
                     for p in range(6)]

            # ---- DMAs: x first (bandwidth priority), then W stages ----
            with ExitStack() as _xs:
                bigp = tc.alloc_tile_pool(name="bigp", bufs=1)
                xtsp = tc.alloc_tile_pool(name="xtsp", bufs=1)

                xts = {}
                for i in range(NT):
                    xt_i = xtsp.tile([128, E], F32, name=f"xt{i}",
                                     tag=f"xt{i}")
                    if i == 0:
                        # split the first tile so PE transposes start sooner
                        for hb in range(2):
                            nc.sync.dma_start(
                                out=xt_i[:, hb * 384:(hb + 1) * 384],
                                in_=x_d[0:128, hb * 384:(hb + 1) * 384])
                    else:
                        nc.sync.dma_start(
                            out=xt_i, in_=x_d[i * 128:(i + 1) * 128, :])
                    xts[i] = xt_i
                for fc in range(12):
                    nc.sync.dma_start(
                        out=bq_cols[fc],
                        in_=bqkv_d[fc * 128:(fc + 1) * 128].rearrange(
                            "(p o) -> p o", o=1))

                # W_qkv Q/K column stages: stage[c] [e-within-j, (j f)] holds
                # W columns c*128:(c+1)*128 for all 6 contraction chunks.
                def stage_dma(dst, col0):
                    nc.gpsimd.dma_start(
                        out=dst.rearrange("p (j f) -> p j f", f=128),
                        in_=wqkv_d[:, col0:col0 + 128].rearrange(
                            "(j p) f -> p j f", p=128))

                stq = [bigp.tile([128, E], BF16, name=f"stq{c}", tag=f"stq{c}")
                       for c in range(6)]
                stk = [bigp.tile([128, E], BF16, name=f"stk{c}", tag=f"stk{c}")
                       for c in range(6)]
                # order: pair 0+1 stages, bv+wqv (for V), then pairs 2..5
                stage_dma(stq[0], 0)
                stage_dma(stk[0], E)
                stage_dma(stq[1], 128)
                stage_dma(stk[1], E + 128)
                bv_row = bigp.tile([1, E], BF16)
                nc.gpsimd.dma_start(
                    out=bv_row,
                    in_=bqkv_d[2 * E:3 * E].rearrange("(o f) -> o f", o=1))
                wqv = [bigp.tile([128, E], BF16, name=f"wqv{j}", tag=f"wqv{j}")
                       for j in range(NE)]
                for j in range(NE):
                    nc.gpsimd.dma_start(
                        out=wqv[j], in_=wqkv_d[j * 128:(j + 1) * 128, 2 * E:])
                for c in range(2, 6):
                    stage_dma(stq[c], c * 128)
                    stage_dma(stk[c], E + c * 128)
                # proj weights last on the SWDGE queue
                wp_sb = [pp.tile([128, E], BF16, name=f"wp{c}", tag=f"wp{c}")
                         for c in range(6)]
                for c in range(6):
                    nc.gpsimd.dma_start(
                        out=wp_sb[c], in_=wproj_d[c * 128:(c + 1) * 128, :])
                bp_row = pp.tile([1, E], BF16)
                nc.gpsimd.dma_start(
                    out=bp_row, in_=bproj_d[:].rearrange("(o f) -> o f", o=1))

                # small psum tags rotate among p1/pa/pz; "tp" is excluded —
                # it persists across a head pair and a QK group allocating it
                # mid-pair would head-block the in-order PE queue on an evac
                # that is emitted later (deadlock).
                _p1rot = ["p1", "pa"]
                _p1i = [0]

                def p1tag():
                    t = _p1rot[_p1i[0] % len(_p1rot)]
                    _p1i[0] += 1
                    return t

                # ---- phase 0: transpose x -> xT (f32r, 1.5 cyc/row) ----
                # groups keyed (i-pair, j-pair) so transposes pipeline behind
                # the x DMA arrivals instead of waiting for all 8 tiles
                xT = [bigp.tile([128, N], BF16, name=f"xT{j}", tag=f"xT{j}")
                      for j in range(NE)]
                for g in range(NT // 2):
                    for jp in range(NE // 2):
                        pt = ps.tile([128, 512], F32, name=f"pt{g}_{jp}",
                                     tag=p1tag(), bufs=1)
                        for dj in range(2):
                            for di in range(2):
                                j, i = 2 * jp + dj, 2 * g + di
                                nc.tensor.transpose(
                                    pt[:, dj * 256 + di * 128:
                                       dj * 256 + (di + 1) * 128],
                                    xts[i][:, j * 128:(j + 1) * 128], identf)
                        for dj in range(2):
                            nc.vector.tensor_copy(
                                xT[2 * jp + dj][:, g * 256:(g + 1) * 256],
                                pt[:, dj * 256:(dj + 1) * 256])
                xtsp.release()

                def emit_QK(c):
                    for dst, stage, bqi in ((qT[c], stq[c], c),
                                            (kT[c], stk[c], 6 + c)):
                        for q in range(NQ):
                            pq = ps.tile([128, 512], F32,
                                         name=f"pq{c}_{bqi}_{q}",
                                         tag=p1tag(), bufs=1)
                            for j in range(NE):
                                nc.tensor.matmul(
                                    pq,
                                    stage[:, j * 128:(j + 1) * 128],
                                    xT[j][:, q * 512:(q + 1) * 512],
                                    start=(j == 0), stop=(j == NE - 1))
                            nc.vector.tensor_scalar_add(
                                dst[:, q * 512:(q + 1) * 512], pq,
                                bq_cols[bqi])

                bv_bc = bigp.tile([128, E], F32)

                def emit_V_bias():
                    for nf, (f0, fw) in enumerate(NFS):
                        pbv = ps.tile([128, 512], F32, name=f"pbv{nf}",
                                      tag=p1tag(), bufs=1)
                        nc.tensor.matmul(pbv[:, :fw], ones1,
                                         bv_row[:, f0:f0 + fw],
                                         start=True, stop=True)
                        nc.vector.tensor_copy(bv_bc[:, f0:f0 + fw],
                                              pbv[:, :fw])

                def emit_V_chunk(i):
                    nc.vector.tensor_copy(
                        vS[i].rearrange("p (h c) -> p h c", c=65)
                            [:, :, 64:65],
                        onesH.rearrange("p (h o) -> p h o", o=1))
                    for nf, (f0, fw) in enumerate(NFS):
                        pv = ps.tile([128, 512], F32, name=f"pv{i}_{nf}",
                                     tag=p1tag(), bufs=1)
                        for j in range(NE):
                            nc.tensor.matmul(
                                pv[:, :fw],
                                xT[j][:, i * 128:(i + 1) * 128],
                                wqv[j][:, f0:f0 + fw],
                                start=(j == 0), stop=(j == NE - 1))
                        nh, h0 = fw // D, f0 // D
                        nc.vector.tensor_add(
                            vS[i].rearrange("p (h c) -> p h c", c=65)
                                [:, h0:h0 + nh, 0:64],
                            pv[:, :fw].rearrange("p (h d) -> p h d", d=D),
                            bv_bc[:, f0:f0 + fw].rearrange(
                                "p (h d) -> p h d", d=D))

                # ---- phase 2 helpers ----
                expS_of = {}
                ptp_of = {}

                def emit_S(h):
                    c, r0 = h // 2, (h % 2) * 64
                    expS = [
                        qp.tile([128, N], BF16, name=f"expS{h}_{kc}",
                                tag="expS", bufs=16)
                        for kc in range(NT)]
                    expS_of[h] = expS
                    for kc in range(NT):
                        pss = ps.tile([128, N], F32, name=f"ps{h}_{kc}",
                                      tag="s", bufs=2)
                        for q in range(NQ):
                            nc.tensor.matmul(
                                pss[:, q * 512:(q + 1) * 512],
                                kT[c][r0:r0 + 64, kc * 128:(kc + 1) * 128],
                                qT[c][r0:r0 + 64, q * 512:(q + 1) * 512],
                                start=True, stop=True)
                        nc.scalar.activation(expS[kc], pss, EXP,
                                             scale=float(SCALE))

                attnS_of = {}
                av_state = {}

                def emit_AV_mm(h, kc_lo, kc_hi):
                    # kc-outer so the AV matmuls chase the exp stream head-on
                    # instead of waiting for all 8 exps of the head
                    if h not in av_state:
                        # AV in cols 0..511 (bank 0), Z in cols 512..519
                        av_state[h] = ps.tile([128, N], F32, name=f"pa{h}",
                                              tag="pa", bufs=1)
                    pa = av_state[h]
                    expS = expS_of[h]
                    for qc in range(NT):
                        for kc in range(kc_lo, kc_hi):
                            nc.tensor.matmul(
                                pa[:, qc * 64:(qc + 1) * 64],
                                expS[kc][:, qc * 128:(qc + 1) * 128],
                                vS[kc][:, h * 65:h * 65 + 64],
                                start=(kc == 0), stop=(kc == NT - 1))
                        for kc in range(kc_lo, kc_hi):
                            nc.tensor.matmul(
                                pa[:, 512 + qc:512 + qc + 1],
                                expS[kc][:, qc * 128:(qc + 1) * 128],
                                vS[kc][:, h * 65 + 64:h * 65 + 65],
                                start=(kc == 0), stop=(kc == NT - 1))

                def emit_AV_fin(h):
                    c, r0 = h // 2, (h % 2) * 64
                    pa = av_state.pop(h)
                    del expS_of[h]
                    rz = qp.tile([128, 8], F32, name=f"rz{h}", tag="rz",
                                 bufs=2)
                    nc.vector.reciprocal(rz, pa[:, 512:512 + NT])
                    # scale into a two-head staging tile; transpose both
                    # heads of the pair at once ([128,128] per q-chunk)
                    if h % 2 == 0:
                        attnS_of[c] = [
                            qp.tile([128, 128], BF16, name=f"as{c}_{qc}",
                                    tag="attnS", bufs=16)
                            for qc in range(NT)]
                    asb = attnS_of[c]
                    for qc in range(NT):
                        nc.vector.tensor_scalar_mul(
                            asb[qc][:, r0:r0 + 64],
                            pa[:, qc * 64:(qc + 1) * 64],
                            rz[:, qc:qc + 1])
                    if h % 2 == 1:
                        ptp = ps.tile([128, N], BF16, name=f"ptp{c}",
                                      tag="tp", bufs=1)
                        for qc in range(NT):
                            nc.tensor.transpose(
                                ptp[:, qc * 128:(qc + 1) * 128],
                                asb[qc], ident_b)
                        if h == H - 1:
                            # half-granular evac so phase 3b starts sooner
                            nc.vector.tensor_copy(attnT[c][:, 0:512],
                                                  ptp[:, 0:512])
                            nc.vector.tensor_copy(attnT[c][:, 512:N],
                                                  ptp[:, 512:N])
                        else:
                            nc.vector.tensor_copy(attnT[c], ptp)
                        del attnS_of[c]

                def emit_AV(h):
                    emit_AV_mm(h, 0, NT)
                    emit_AV_fin(h)

                # ---- interleaved schedule ----
                # exp(h) chains on ACT; PE stays ahead: S(h+1) before AV(h),
                # QK(c+1)/V slotted into the attention stream.
                emit_QK(0)
                emit_S(0)
                emit_S(1)
                emit_QK(1)
                emit_V_bias()
                for i in range(NT):
                    emit_V_chunk(i)
                emit_AV(0)
                emit_S(2)
                emit_AV(1)
                emit_QK(2)
                emit_S(3)
                emit_AV(2)
                emit_S(4)
                emit_AV(3)
                emit_QK(3)
                emit_S(5)
                emit_AV(4)
                emit_S(6)
                emit_AV(5)
                emit_QK(4)
                # broadcast b_proj while the attention stream has slack
                bp_bc = pp.tile([128, E], F32)
                for nf, (f0, fw) in enumerate(NFS):
                    pbp = ps.tile([128, 512], F32, name=f"pbp{nf}",
                                  tag=p1tag(), bufs=1)
                    nc.tensor.matmul(pbp[:, :fw], ones1, bp_row[:, f0:f0 + fw],
                                     start=True, stop=True)
                    nc.vector.tensor_copy(bp_bc[:, f0:f0 + fw], pbp[:, :fw])
                emit_S(7)
                emit_AV(6)
                emit_S(8)
                emit_AV(7)
                emit_QK(5)
                emit_S(9)
                emit_AV(8)
                emit_S(10)
                emit_AV(9)
                emit_S(11)
                emit_AV(10)
                emit_AV(11)
                bigp.release()

                # ---- phase 3: output projection ----
                osbp = tc.alloc_tile_pool(name="osb", bufs=1)
                _xs.callback(lambda: osbp.release())
                for i in range(NT):
                    o_sb = osbp.tile([128, E], F32, name=f"o{i}", tag="o",
                                     bufs=4)
                    for nf, (f0, fw) in enumerate(NFS):
                        po = ps.tile([128, 512], F32, name=f"po{i}_{nf}",
                                     tag=("s" if nf == 0 else "pa"),
                                     bufs=(2 if nf == 0 else 1))
                        for c in range(6):
                            nc.tensor.matmul(
                                po[:, :fw],
                                attnT[c][:, i * 128:(i + 1) * 128],
                                wp_sb[c][:, f0:f0 + fw],
                                start=(c == 0), stop=(c == 5))
                        nc.vector.tensor_add(
                            o_sb[:, f0:f0 + fw], po[:, :fw],
                            bp_bc[:, f0:f0 + fw])
                        if i == NT - 1:
                            nc.sync.dma_start(
                                out=out_d[i * 128:(i + 1) * 128, f0:f0 + fw],
                                in_=o_sb[:, f0:f0 + fw])
                    if i < NT - 1:
                        nc.sync.dma_start(
                            out=out_d[i * 128:(i + 1) * 128, :], in_=o_sb)
    nc.compile()
    return nc


_NC_CACHE = None


def kernel(x, W_qkv, b_qkv, W_proj, b_proj):
    from concourse.bass_utils import run_bass_kernel_spmd

    global _NC_CACHE
    if _NC_CACHE is None:
        _NC_CACHE = _build()
    nc = _NC_CACHE

    x = np.ascontiguousarray(np.asarray(x, dtype=np.float32))
    W_qkv = np.ascontiguousarray(np.asarray(W_qkv, dtype=np.float32))
    b_qkv = np.ascontiguousarray(np.asarray(b_qkv, dtype=np.float32))
    W_proj = np.ascontiguousarray(np.asarray(W_proj, dtype=np.float32))
    b_proj = np.ascontiguousarray(np.asarray(b_proj, dtype=np.float32))

    in_maps = [
        {"x": x[b], "W_qkv": W_qkv, "b_qkv": b_qkv,
         "W_proj": W_proj, "b_proj": b_proj}
        for b in range(B)
    ]
    res = run_bass_kernel_spmd(nc, in_maps, core_ids=list(range(B)))
    return np.stack([np.asarray(res.results[b]["out"]) for b in range(B)])


# revision 36
# speedup vs baseline: 1.1115x; 1.0230x over previous
"""Multi-head attention block on 8 Trainium2 NeuronCores.

Problem: B=8, N=1024, E=768, H=12, D=64 attention (QKV proj -> softmax(QK^T/8)V
-> output proj), fp32 I/O.

Sharding: data parallel over batch — core b computes batch element b entirely
locally; no collectives. Host shards/stacks.

Per-core kernel v2 (ACT-exp-chain saturated; PE work minimized):
  phase 0: DMA x -> PE-transpose (f32r, 8 per 2-bank psum) -> xT [E, N]
  phase 1 (interleaved with phase 2 per head-pair):
    V natural vS[i] [128, 65*12] bf16 with a ones column per head (col 65h+64)
    qT/kT pairs [128, N] bf16: rows (h%2)*64 hold head h's d-dims; W_qkv
    columns DMA'd as per-pair "stage" tiles [e-part, (j f)] so QK(c) can
    start as soon as its two stages land.
  phase 2 per head: S^T[k,q] psum via 2 512-wide matmuls (bf16);
    exp on ACT (scale=1/8) -> expS bf16; AV: stat=expS chunk, mov=[V] 64-wide
    bf16 accumulating over kc -> attn natural [q,64] psum; Z via 1-wide ones
    matmuls -> pz; invZ=recip(pz); DVE scale -> attnS bf16; PE transpose
    (bf16 identity, 1cyc/row) -> attnT pair psum; DVE evac per pair.
  phase 3: out[t, e] = sum_c attnT[c].T W_proj[c] + b_proj (f32r)
"""
import numpy as np

B, N, E, H, D = 8, 1024, 768, 12, 64
SCALE = D ** -0.5
NT = N // 128   # token chunks (8)
NE = E // 128   # embed chunks (6)
NQ = N // 512   # moving-dim tiles (2)
NFS = [(0, 512), (512, 256)]  # free-dim split of E for matmuls


def _build():
    import concourse.bacc as bacc
    import concourse.mybir as mybir
    import concourse.tile as tile
    from concourse.masks import make_identity
    from contextlib import ExitStack

    F32 = mybir.dt.float32
    F32R = mybir.dt.float32r
    BF16 = mybir.dt.bfloat16
    EXP = mybir.ActivationFunctionType.Exp

    nc = bacc.Bacc("TRN2", target_bir_lowering=False)
    x_d = nc.declare_dram_parameter("x", [N, E], F32, isOutput=False)
    wqkv_d = nc.declare_dram_parameter("W_qkv", [E, 3 * E], F32, isOutput=False)
    bqkv_d = nc.declare_dram_parameter("b_qkv", [3 * E], F32, isOutput=False)
    wproj_d = nc.declare_dram_parameter("W_proj", [E, E], F32, isOutput=False)
    bproj_d = nc.declare_dram_parameter("b_proj", [E], F32, isOutput=False)
    out_d = nc.declare_dram_parameter("out", [N, E], F32, isOutput=True)

    with tile.TileContext(nc) as tc:
        with (
            tc.tile_pool(name="const", bufs=1) as cp,
            tc.tile_pool(name="qkv", bufs=1) as qp,
            tc.tile_pool(name="psum", bufs=1, space="PSUM") as ps,
            tc.tile_pool(name="proj", bufs=1) as pp,
        ):
            # ---- constants ----
            identf = cp.tile([128, 128], F32)
            make_identity(nc, identf)
            ident_b = cp.tile([128, 128], BF16)
            nc.vector.tensor_copy(ident_b, identf)
            ones1 = cp.tile([1, 128], BF16)
            nc.vector.memset(ones1, 1.0)
            onesH = cp.tile([128, H], BF16)
            nc.vector.memset(onesH, 1.0)
            bq_cols = [cp.tile([128, 1], F32, name=f"bq_{fc}", tag=f"bq_{fc}")
                       for fc in range(12)]

            # ---- long-lived attention-layout tensors ----
            qT = [qp.tile([128, N], BF16, name=f"qT{c}", tag=f"qT{c}")
                  for c in range(6)]
            kT = [qp.tile([128, N], BF16, name=f"kT{c}", tag=f"kT{c}")
                  for c in range(6)]
            vS = [qp.tile([128, 65 * H], BF16, name=f"vS{i}", tag=f"vS{i}")
                  for i in range(NT)]
            attnT = [qp.tile([128, N], BF16, name=f"attnT{p}", tag=f"attnT{p}")
                     for p in range(6)]

            # ---- DMAs: x first (bandwidth priority), then W stages ----
            with ExitStack() as _xs:
                bigp = tc.alloc_tile_pool(name="bigp", bufs=1)
                xtsp = tc.alloc_tile_pool(name="xtsp", bufs=1)

                xts = {}
                for i in range(NT):
                    xt_i = xtsp.tile([128, E], F32, name=f"xt{i}",
                                     tag=f"xt{i}")
                    if i == 0:
                        # split the first tile so PE transposes start sooner
                        for hb in range(2):
                            nc.sync.dma_start(
                                out=xt_i[:, hb * 384:(hb + 1) * 384],
                                in_=x_d[0:128, hb * 384:(hb + 1) * 384])
                    else:
                        nc.sync.dma_start(
                            out=xt_i, in_=x_d[i * 128:(i + 1) * 128, :])
                    xts[i] = xt_i
                for fc in range(12):
                    nc.sync.dma_start(
                        out=bq_cols[fc],
                        in_=bqkv_d[fc * 128:(fc + 1) * 128].rearrange(
                            "(p o) -> p o", o=1))

                # W_qkv Q/K column stages: stage[c] [e-within-j, (j f)] holds
                # W columns c*128:(c+1)*128 for all 6 contraction chunks.
                def stage_dma(dst, col0):
                    nc.gpsimd.dma_start(
                        out=dst.rearrange("p (j f) -> p j f", f=128),
                        in_=wqkv_d[:, col0:col0 + 128].rearrange(
                            "(j p) f -> p j f", p=128))

                stq = [bigp.tile([128, E], BF16, name=f"stq{c}", tag=f"stq{c}")
                       for c in range(6)]
                stk = [bigp.tile([128, E], BF16, name=f"stk{c}", tag=f"stk{c}")
                       for c in range(6)]
                # order: pair 0+1 stages, bv+wqv (for V), then pairs 2..5
                stage_dma(stq[0], 0)
                stage_dma(stk[0], E)
                stage_dma(stq[1], 128)
                stage_dma(stk[1], E + 128)
                bv_row = bigp.tile([1, E], BF16)
                nc.gpsimd.dma_start(
                    out=bv_row,
                    in_=bqkv_d[2 * E:3 * E].rearrange("(o f) -> o f", o=1))
                wqv = [bigp.tile([128, E], BF16, name=f"wqv{j}", tag=f"wqv{j}")
                       for j in range(NE)]
                for j in range(NE):
                    nc.gpsimd.dma_start(
                        out=wqv[j], in_=wqkv_d[j * 128:(j + 1) * 128, 2 * E:])
                for c in range(2, 6):
                    stage_dma(stq[c], c * 128)
                    stage_dma(stk[c], E + c * 128)
                # proj weights last on the SWDGE queue
                wp_sb = [pp.tile([128, E], BF16, name=f"wp{c}", tag=f"wp{c}")
                         for c in range(6)]
                for c in range(6):
                    nc.gpsimd.dma_start(
                        out=wp_sb[c], in_=wproj_d[c * 128:(c + 1) * 128, :])
                bp_row = pp.tile([1, E], BF16)
                nc.gpsimd.dma_start(
                    out=bp_row, in_=bproj_d[:].rearrange("(o f) -> o f", o=1))

                # small psum tags rotate among p1/pa/pz; "tp" is excluded —
                # it persists across a head pair and a QK group allocating it
                # mid-pair would head-block the in-order PE queue on an evac
                # that is emitted later (deadlock).
                _p1rot = ["p1", "pa"]
                _p1i = [0]

                def p1tag():
                    t = _p1rot[_p1i[0] % len(_p1rot)]
                    _p1i[0] += 1
                    return t

                # ---- phase 0: transpose x -> xT (f32r, 1.5 cyc/row) ----
                # groups keyed (i-pair, j-pair) so transposes pipeline behind
                # the x DMA arrivals instead of waiting for all 8 tiles
                xT = [bigp.tile([128, N], BF16, name=f"xT{j}", tag=f"xT{j}")
                      for j in range(NE)]
                for g in range(NT // 2):
                    for jp in range(NE // 2):
                        pt = ps.tile([128, 512], F32, name=f"pt{g}_{jp}",
                                     tag=p1tag(), bufs=1)
                        for dj in range(2):
                            for di in range(2):
                                j, i = 2 * jp + dj, 2 * g + di
                                nc.tensor.transpose(
                                    pt[:, dj * 256 + di * 128:
                                       dj * 256 + (di + 1) * 128],
                                    xts[i][:, j * 128:(j + 1) * 128], identf)
                        for dj in range(2):
                            nc.vector.tensor_copy(
                                xT[2 * jp + dj][:, g * 256:(g + 1) * 256],
                                pt[:, dj * 256:(dj + 1) * 256])
                xtsp.release()

                def emit_QK(c):
                    for dst, stage, bqi in ((qT[c], stq[c], c),
                                            (kT[c], stk[c], 6 + c)):
                        for q in range(NQ):
                            pq = ps.tile([128, 512], F32,
                                         name=f"pq{c}_{bqi}_{q}",
                                         tag=p1tag(), bufs=1)
                            for j in range(NE):
                                nc.tensor.matmul(
                                    pq,
                                    stage[:, j * 128:(j + 1) * 128],
                                    xT[j][:, q * 512:(q + 1) * 512],
                                    start=(j == 0), stop=(j == NE - 1))
                            nc.vector.tensor_scalar_add(
                                dst[:, q * 512:(q + 1) * 512], pq,
                                bq_cols[bqi])

                bv_bc = bigp.tile([128, E], F32)

                def emit_V_bias():
                    for nf, (f0, fw) in enumerate(NFS):
                        pbv = ps.tile([128, 512], F32, name=f"pbv{nf}",
                                      tag=p1tag(), bufs=1)
                        nc.tensor.matmul(pbv[:, :fw], ones1,
                                         bv_row[:, f0:f0 + fw],
                                         start=True, stop=True)
                        nc.vector.tensor_copy(bv_bc[:, f0:f0 + fw],
                                              pbv[:, :fw])

                def emit_V_chunk(i):
                    nc.vector.tensor_copy(
                        vS[i].rearrange("p (h c) -> p h c", c=65)
                            [:, :, 64:65],
                        onesH.rearrange("p (h o) -> p h o", o=1))
                    for nf, (f0, fw) in enumerate(NFS):
                        pv = ps.tile([128, 512], F32, name=f"pv{i}_{nf}",
                                     tag=p1tag(), bufs=1)
                        for j in range(NE):
                            nc.tensor.matmul(
                                pv[:, :fw],
                                xT[j][:, i * 128:(i + 1) * 128],
                                wqv[j][:, f0:f0 + fw],
                                start=(j == 0), stop=(j == NE - 1))
                        nh, h0 = fw // D, f0 // D
                        nc.vector.tensor_add(
                            vS[i].rearrange("p (h c) -> p h c", c=65)
                                [:, h0:h0 + nh, 0:64],
                            pv[:, :fw].rearrange("p (h d) -> p h d", d=D),
                            bv_bc[:, f0:f0 + fw].rearrange(
                                "p (h d) -> p h d", d=D))

                # ---- phase 2 helpers ----
                expS_of = {}
                ptp_of = {}

                def emit_S(h):
                    c, r0 = h // 2, (h % 2) * 64
                    expS = [
                        qp.tile([128, N], BF16, name=f"expS{h}_{kc}",
                                tag="expS", bufs=24)
                        for kc in range(NT)]
                    expS_of[h] = expS
                    for kc in range(NT):
                        pss = ps.tile([128, N], F32, name=f"ps{h}_{kc}",
                                      tag="s", bufs=2)
                        for q in range(NQ):
                            nc.tensor.matmul(
                                pss[:, q * 512:(q + 1) * 512],
                                kT[c][r0:r0 + 64, kc * 128:(kc + 1) * 128],
                                qT[c][r0:r0 + 64, q * 512:(q + 1) * 512],
                                start=True, stop=True)
                        nc.scalar.activation(expS[kc], pss, EXP,
                                             scale=float(SCALE))

                attnS_of = {}
                av_state = {}

                def emit_AV_mm(h, kc_lo, kc_hi):
                    # kc-outer so the AV matmuls chase the exp stream head-on
                    # instead of waiting for all 8 exps of the head
                    if h not in av_state:
                        # AV in cols 0..511 (bank 0), Z in cols 512..519
                        av_state[h] = ps.tile([128, N], F32, name=f"pa{h}",
                                              tag="pa", bufs=1)
                    pa = av_state[h]
                    expS = expS_of[h]
                    for qc in range(NT):
                        for kc in range(kc_lo, kc_hi):
                            nc.tensor.matmul(
                                pa[:, qc * 64:(qc + 1) * 64],
                                expS[kc][:, qc * 128:(qc + 1) * 128],
                                vS[kc][:, h * 65:h * 65 + 64],
                                start=(kc == 0), stop=(kc == NT - 1))
                        for kc in range(kc_lo, kc_hi):
                            nc.tensor.matmul(
                                pa[:, 512 + qc:512 + qc + 1],
                                expS[kc][:, qc * 128:(qc + 1) * 128],
                                vS[kc][:, h * 65 + 64:h * 65 + 65],
                                start=(kc == 0), stop=(kc == NT - 1))

                def emit_AV_fin(h):
                    c, r0 = h // 2, (h % 2) * 64
                    pa = av_state.pop(h)
                    del expS_of[h]
                    rz = qp.tile([128, 8], F32, name=f"rz{h}", tag="rz",
                                 bufs=2)
                    nc.vector.reciprocal(rz, pa[:, 512:512 + NT])
                    # scale into a two-head staging tile; transpose both
                    # heads of the pair at once ([128,128] per q-chunk)
                    if h % 2 == 0:
                        attnS_of[c] = [
                            qp.tile([128, 128], BF16, name=f"as{c}_{qc}",
                                    tag="attnS", bufs=16)
                            for qc in range(NT)]
                    asb = attnS_of[c]
                    for qc in range(NT):
                        nc.vector.tensor_scalar_mul(
                            asb[qc][:, r0:r0 + 64],
                            pa[:, qc * 64:(qc + 1) * 64],
                            rz[:, qc:qc + 1])
                    if h % 2 == 1:
                        ptp = ps.tile([128, N], BF16, name=f"ptp{c}",
                                      tag="tp", bufs=1)
                        for qc in range(NT):
                            nc.tensor.transpose(
                                ptp[:, qc * 128:(qc + 1) * 128],
                                asb[qc], ident_b)
                        if h == H - 1:
                            # half-granular evac so phase 3b starts sooner
                            nc.vector.tensor_copy(attnT[c][:, 0:512],
                                                  ptp[:, 0:512])
                            nc.vector.tensor_copy(attnT[c][:, 512:N],
                                                  ptp[:, 512:N])
                        else:
                            nc.vector.tensor_copy(attnT[c], ptp)
                        del attnS_of[c]

                def emit_AV(h):
                    emit_AV_mm(h, 0, NT)
                    emit_AV_fin(h)

                # ---- interleaved schedule ----
                # exp(h) chains on ACT; PE stays ahead: S(h+1) before AV(h),
                # QK(c+1)/V slotted into the attention stream.
                emit_QK(0)
                emit_S(0)
                emit_S(1)
                emit_QK(1)
                emit_V_bias()
                for i in range(NT):
                    emit_V_chunk(i)
                emit_AV(0)
                emit_S(2)
                emit_AV(1)
                emit_QK(2)
                emit_S(3)
                emit_AV(2)
                emit_S(4)
                emit_AV(3)
                emit_QK(3)
                emit_S(5)
                emit_AV(4)
                emit_S(6)
                emit_AV(5)
                emit_QK(4)
                # broadcast b_proj while the attention stream has slack
                bp_bc = pp.tile([128, E], F32)
                for nf, (f0, fw) in enumerate(NFS):
                    pbp = ps.tile([128, 512], F32, name=f"pbp{nf}",
                                  tag=p1tag(), bufs=1)
                    nc.tensor.matmul(pbp[:, :fw], ones1, bp_row[:, f0:f0 + fw],
                                     start=True, stop=True)
                    nc.vector.tensor_copy(bp_bc[:, f0:f0 + fw], pbp[:, :fw])
                emit_S(7)
                emit_AV(6)
                emit_S(8)
                emit_AV(7)
                emit_QK(5)
                emit_S(9)
                emit_AV(8)
                emit_S(10)
                emit_AV(9)
                emit_S(11)
                emit_AV(10)
                emit_AV(11)
                bigp.release()

                # ---- phase 3: output projection ----
                osbp = tc.alloc_tile_pool(name="osb", bufs=1)
                _xs.callback(lambda: osbp.release())
                for i in range(NT):
                    o_sb = osbp.tile([128, E], F32, name=f"o{i}", tag="o",
                                     bufs=4)
                    for nf, (f0, fw) in enumerate(NFS):
                        po = ps.tile([128, 512], F32, name=f"po{i}_{nf}",
                                     tag=("s" if nf == 0 else "pa"),
                                     bufs=(2 if nf == 0 else 1))
                        for c in range(6):
                            nc.tensor.matmul(
                                po[:, :fw],
                                attnT[c][:, i * 128:(i + 1) * 128],
                                wp_sb[c][:, f0:f0 + fw],
                                start=(c == 0), stop=(c == 5))
                        nc.vector.tensor_add(
                            o_sb[:, f0:f0 + fw], po[:, :fw],
                            bp_bc[:, f0:f0 + fw])
                        if i == NT - 1:
                            nc.sync.dma_start(
                                out=out_d[i * 128:(i + 1) * 128, f0:f0 + fw],
                                in_=o_sb[:, f0:f0 + fw])
                    if i < NT - 1:
                        nc.sync.dma_start(
                            out=out_d[i * 128:(i + 1) * 128, :], in_=o_sb)
    nc.compile()
    return nc


_NC_CACHE = None


def kernel(x, W_qkv, b_qkv, W_proj, b_proj):
    from concourse.bass_utils import run_bass_kernel_spmd

    global _NC_CACHE
    if _NC_CACHE is None:
        _NC_CACHE = _build()
    nc = _NC_CACHE

    x = np.ascontiguousarray(np.asarray(x, dtype=np.float32))
    W_qkv = np.ascontiguousarray(np.asarray(W_qkv, dtype=np.float32))
    b_qkv = np.ascontiguousarray(np.asarray(b_qkv, dtype=np.float32))
    W_proj = np.ascontiguousarray(np.asarray(W_proj, dtype=np.float32))
    b_proj = np.ascontiguousarray(np.asarray(b_proj, dtype=np.float32))

    in_maps = [
        {"x": x[b], "W_qkv": W_qkv, "b_qkv": b_qkv,
         "W_proj": W_proj, "b_proj": b_proj}
        for b in range(B)
    ]
    res = run_bass_kernel_spmd(nc, in_maps, core_ids=list(range(B)))
    return np.stack([np.asarray(res.results[b]["out"]) for b in range(B)])
